# revision 18
# baseline (speedup 1.0000x reference)
"""Trainium2 (8 NeuronCores, Bass/Tile) kernel for
nn_AdaptiveDiscriminatorAugmentation.

Strategy
--------
Pure data parallel: 32 samples -> 8 cores x 4 sample positions, assigned by a
grouping optimizer so per-position warp templates minimize total slot cost.
The integer permutation stages (flip / rot90 / roll) are applied while
assembling the per-core input shards (pure index permutations, part of the
same host copy that builds the shards).  The resampling stages run on device
as a uniform SPMD program; all per-sample variation (which scale/angle,
identity padding) is carried as data:

- separable warps (scale1 / scale2 / identity padding) = two TensorEngine
  one-hot matmul passes with host-built bf16 hat matrices:
      pass1: T1[c, y] = sum_r I[r, c] * V[r, y]   (stationary = image tile)
      pass2: out[y, x] = sum_c T1[c, y] * H[c, x] (stationary = T1 tile)
  no transposes needed: pass1 emits column-major, pass2 restores row-major.
  PSUM->SBUF copies are merged 4 channels at a time and alternate between
  the Scalar and Vector engines.
- rotation warps = per-pixel bilinear gather:
      relayout (Scalar) -> blocked HBM scratch write (DMA) -> dma_gather
      (512B elems = 2 rows x 4px x 32ch bf16, int16 idx = (c0>>1)*256+r0,
      4 gathers/chunk round-robined over 4 SWDGE queues so descriptor
      generation parallelizes) -> 11-op DVE blend with 6 weight fields
      (x-parity folded into the slot weights) materialized per chunk on the
      Scalar engine so every blend operand has unit inner stride.
  dma_gather is limited to 1024 indices per call (SWDGE ring capacity).

Per-position slot templates are minimal-cost subsequences of [P, G, P, G]
(= scale1, rot1, scale2, rot2); a local-search optimizer assigns samples to
positions to minimize total gather (G) and matmul (P) slots; padding slots
run exact identities (bf16-exact hat/weight values).
"""
import math
import os
import sys

import numpy as np

sys.path.insert(0, "/opt/trn_rl_repo")

H = W = 256
C = 24
CPAD = 32
NPIX = H * W
CHUNK_COLS = 16
CHUNK_PIX = CHUNK_COLS * H
N_CHUNKS = W // CHUNK_COLS
N_CORES = 8
SAMPLES_PER_CORE = 4
SCRATCH_ELEMS = 128 * 256 * 128
GSUB = 512                       # dma_gather calls sized so two fit per SWDGE queue ring

_xs = np.arange(W)
I_OF_YX = ((_xs // CHUNK_COLS)[None, :] * CHUNK_PIX
           + (_xs % CHUNK_COLS)[None, :] * H + np.arange(H)[:, None])


# ---------------------------------------------------------------------------
# host-side warp math

def _warp_coords(angle_deg, sy, sx):
    cy = cx = 0.5 * H
    th = angle_deg * (math.pi / 180.0)
    cos, sin = np.cos(th), np.sin(th)
    ys, xs = np.meshgrid(np.arange(H, dtype=np.float64),
                         np.arange(W, dtype=np.float64), indexing="ij")
    dy, dx = ys - cy, xs - cx
    xin = (cos * dx + sin * dy) / sx + cx
    yin = (-sin * dx + cos * dy) / sy + cy
    m = float(H - 1)

    def reflect(c):
        c = np.abs(c) % (2.0 * m)
        return np.minimum(c, 2.0 * m - c)

    return reflect(yin), reflect(xin)


def _axis_coords(scale):
    """1D inverse-map + reflect for a separable (axis-aligned) warp."""
    cy = 0.5 * H
    v = (np.arange(H, dtype=np.float64) - cy) / scale + cy
    m = float(H - 1)
    v = np.abs(v) % (2.0 * m)
    return np.minimum(v, 2.0 * m - v)


def _hat_matrix(v):
    """[256 src, 256 dst] bilinear hat weights for dst coords v (float64)."""
    r0 = np.minimum(np.floor(v), 254.0).astype(np.int64)
    w = (v - r0).astype(np.float32)
    M = np.zeros((H, H), np.float32)
    j = np.arange(H)
    M[r0, j] += 1.0 - w
    M[r0 + 1, j] += w
    return M


def _sample_slots(m_s1, s1, m_r1, a1, m_s2, s2, m_r2, a2):
    """Slot list for one sample: ("P", (vy, vx)) or ("G", (yin, xin)).
    Only the stages the sample actually needs (order: s1, r1, s2, r2)."""
    slots = []
    if m_s1:
        slots.append(("P", (_axis_coords(s1), _axis_coords(s1))))
    if m_r1:
        slots.append(("G", _warp_coords(a1, 1.0, 1.0)))
    if m_s2:
        slots.append(("P", (_axis_coords(s2[0]), _axis_coords(s2[1]))))
    if m_r2:
        slots.append(("G", _warp_coords(a2, 1.0, 1.0)))
    return slots


def _gather_arrays(yin, xin):
    """(idx [128, NPIX//16] int16, W [128, 6, NPIX//128] float32)."""
    ii = np.arange(NPIX)
    yin_f = np.empty(NPIX)
    xin_f = np.empty(NPIX)
    yin_f[I_OF_YX.ravel()] = yin.ravel()
    xin_f[I_OF_YX.ravel()] = xin.ravel()
    r0 = np.minimum(np.floor(yin_f), 254.0)
    c0 = np.minimum(np.floor(xin_f), 254.0)
    wy = (yin_f - r0).astype(np.float32)
    wx = (xin_f - c0).astype(np.float32)
    par = (c0.astype(np.int64) & 1).astype(np.float32)
    idx = (c0.astype(np.int64) >> 1) * 256 + r0.astype(np.int64)
    assert 0 <= idx.min() and idx.max() <= 32766
    idx_w = np.zeros((16, NPIX // 16), np.int16)
    idx_w[ii % 16, ii // 16] = idx.astype(np.int16)
    idx_wrapped = np.tile(idx_w, (8, 1))
    s0 = (1 - par) * (1 - wx)
    s1 = par * (1 - wx) + (1 - par) * wx
    s2 = par * wx
    v0, v1 = (1 - wy), wy
    Wf = np.stack([v0 * s0, v0 * s1, v0 * s2, v1 * s0, v1 * s1, v1 * s2])
    Wdev = np.zeros((128, 6, NPIX // 128), np.float32)
    Wdev[ii % 128, :, ii // 128] = Wf.astype(np.float32).T
    return idx_wrapped, Wdev


def _pe_arrays(vy, vx):
    """vh [2, 128, 2, 256] float32: [0] = V[r, y], [1] = H[c, x];
    partition-major p = src & 127, rt = src >> 7."""
    V = _hat_matrix(vy)
    Hm = _hat_matrix(vx)
    out = np.zeros((2, 128, 2, 256), np.float32)
    src = np.arange(H)
    out[0, src & 127, src >> 7, :] = V
    out[1, src & 127, src >> 7, :] = Hm
    return out


_ID_G = None
_ID_P = None


def _identity_g():
    global _ID_G
    if _ID_G is None:
        ys, xs = np.meshgrid(np.arange(H, dtype=np.float64),
                             np.arange(W, dtype=np.float64), indexing="ij")
        _ID_G = _gather_arrays(ys, xs)
    return _ID_G


def _identity_p():
    global _ID_P
    if _ID_P is None:
        ident = np.arange(H, dtype=np.float64)
        _ID_P = _pe_arrays(ident, ident)
    return _ID_P


# ---------------------------------------------------------------------------
# grouping optimizer: assign samples to the 4 positions so the merged
# per-position templates minimize total slot cost.

_COST_G = 240
_COST_P = 100
_TPOS = ("P", "G", "P", "G")


def _embeds(seq, t):
    """Greedy subsequence embedding of a sample slot-type seq into template t."""
    j = 0
    for ty in t:
        if j < len(seq) and seq[j] == ty:
            j += 1
    return j == len(seq)


def _min_template(seqs):
    best, bc = None, None
    for mask in range(16):
        t = tuple(_TPOS[k] for k in range(4) if (mask >> k) & 1)
        if all(_embeds(s, t) for s in seqs):
            c = sum(_COST_G if ty == "G" else _COST_P for ty in t)
            if bc is None or c < bc:
                best, bc = t, c
    assert best is not None
    return best, bc


def _optimize_groups(seqs_all):
    B = len(seqs_all)
    order = sorted(range(B),
                   key=lambda i: (seqs_all[i].count("G"), seqs_all[i]),
                   reverse=True)
    groups = [order[p * N_CORES:(p + 1) * N_CORES]
              for p in range(SAMPLES_PER_CORE)]

    def gcost(g):
        return _min_template([seqs_all[i] for i in g])[1]

    costs = [gcost(g) for g in groups]
    improved = True
    while improved:
        improved = False
        for a in range(SAMPLES_PER_CORE):
            for b in range(a + 1, SAMPLES_PER_CORE):
                for ia in range(N_CORES):
                    for ib in range(N_CORES):
                        ga = groups[a][:]
                        gb = groups[b][:]
                        ga[ia], gb[ib] = gb[ib], ga[ia]
                        ca, cb = gcost(ga), gcost(gb)
                        if ca + cb < costs[a] + costs[b]:
                            groups[a], groups[b] = ga, gb
                            costs[a], costs[b] = ca, cb
                            improved = True
    # heaviest positions first so their gathers start early
    order_p = sorted(range(SAMPLES_PER_CORE), key=lambda p: -costs[p])
    groups = [groups[p] for p in order_p]
    return groups


# ---------------------------------------------------------------------------
# device program

_GRAPH_CACHE = {}


def _build_graph(template):
    """template: tuple per position of slot-type strings, e.g.
    (('P','G','P','G'), ('P','G'), ('P',), ('P',))."""
    import concourse.bacc as bacc
    import concourse.mybir as mybir
    import bass_rust
    from concourse.tile import TileContext
    from concourse.library_config import mlp

    dt = mybir.dt
    Alu = mybir.AluOpType
    NG = sum(t.count("G") for t in template)
    NP_ = sum(t.count("P") for t in template)

    nc = bacc.Bacc("TRN2", num_swdge_queues=4)
    img_in = nc.declare_dram_parameter(
        "img", [SAMPLES_PER_CORE, C, H, W], dt.bfloat16, isOutput=False)
    idx_in = nc.declare_dram_parameter(
        "idx", [max(NG, 1), 128, NPIX // 16], dt.int16, isOutput=False)
    wgt_in = nc.declare_dram_parameter(
        "wgt", [max(NG, 1), 128, 6, NPIX // 128], dt.bfloat16, isOutput=False)
    vh_in = nc.declare_dram_parameter(
        "vh", [max(NP_, 1), 2, 128, 2, 256], dt.bfloat16, isOutput=False)
    out_t = nc.declare_dram_parameter(
        "out", [SAMPLES_PER_CORE, C, H, W], dt.float32, isOutput=True)

    copy_tick = 0

    with TileContext(nc) as tc:
        nc.gpsimd.load_library(mlp)
        with (tc.tile_pool(name="scrp", bufs=2, space="DRAM") as scr_pool,
              tc.tile_pool(name="psum", bufs=4, space="PSUM") as psum_pool,
              tc.tile_pool(name="canon", bufs=2) as canon_pool,
              tc.tile_pool(name="stage", bufs=2) as stage_pool,
              tc.tile_pool(name="rowmaj", bufs=1) as rowmaj_pool,
              tc.tile_pool(name="gbuf", bufs=2) as gbuf_pool,
              tc.tile_pool(name="idxp", bufs=2) as idx_pool,
              tc.tile_pool(name="wgtp", bufs=2) as wgt_pool,
              tc.tile_pool(name="wexp", bufs=2) as wexp_pool,
              tc.tile_pool(name="t1p", bufs=1) as t1_pool,
              tc.tile_pool(name="vhp", bufs=2) as vh_pool,
              tc.tile_pool(name="btmp", bufs=1) as btmp_pool):

            def merged_copy(out_ap, ps_ap):
                """PSUM->SBUF copy, alternating Scalar / Vector."""
                nonlocal copy_tick
                if copy_tick % 2 == 0:
                    nc.scalar.copy(out=out_ap, in_=ps_ap)
                else:
                    nc.vector.tensor_copy(out_ap, ps_ap)
                copy_tick += 1

            canons = {}

            def emit_load(s):
                canon = canon_pool.tile([128, 2, C, W], dt.bfloat16,
                                        name="canon")
                canons[s] = canon
                # ---- load bf16 (host-cast) straight into canonical
                # [p=y&127, yt, c, x] -- no stage tile, no cast op
                for yt in range(2):
                    for ch in range(4):
                        nc.sync.dma_start(
                            out=canon[:, yt, 6 * ch:6 * (ch + 1), :],
                            in_=img_in[s, 6 * ch:6 * (ch + 1),
                                       128 * yt:128 * (yt + 1), :]
                            .rearrange("c p x -> p c x"))

            def emit_slot(s, ty, slot_id):
                canon = canons[s]
                if ty == "P":
                    p_slot = slot_id
                    vh = vh_pool.tile([128, 2, 2, 256], dt.bfloat16,
                                      name="vh")
                    nc.sync.dma_start(
                        out=vh[:, :, :, :],
                        in_=vh_in[p_slot].rearrange("w p rt f -> p w rt f"))
                    t1 = t1_pool.tile([128, 2, C, 256], dt.bfloat16,
                                      name="t1")
                    # pass 1: T1[c, y] = sum_r I[r, c] V[r, y]
                    for ct in range(2):
                        for q in range(C // 4):
                            ps = psum_pool.tile([128, 1024], dt.float32,
                                                name="ps")
                            for k in range(4):
                                ch = 4 * q + k
                                for rt in range(2):
                                    nc.tensor.matmul(
                                        ps[:, 256 * k:256 * (k + 1)],
                                        canon[:, rt, ch,
                                              128 * ct:128 * (ct + 1)],
                                        vh[:, 0, rt, :],
                                        start=(rt == 0), stop=(rt == 1))
                            merged_copy(
                                t1[:, ct, 4 * q:4 * (q + 1), :],
                                ps[:, :].rearrange("p (c y) -> p c y", c=4))
                    # pass 2: out[y, x] = sum_c T1[c, y] H[c, x]
                    for yt in range(2):
                        for q in range(C // 4):
                            ps2 = psum_pool.tile([128, 1024], dt.float32,
                                                 name="ps")
                            for k in range(4):
                                ch = 4 * q + k
                                for ct in range(2):
                                    nc.tensor.matmul(
                                        ps2[:, 256 * k:256 * (k + 1)],
                                        t1[:, ct, ch,
                                           128 * yt:128 * (yt + 1)],
                                        vh[:, 1, ct, :],
                                        start=(ct == 0), stop=(ct == 1))
                            merged_copy(
                                canon[:, yt, 4 * q:4 * (q + 1), :],
                                ps2[:, :].rearrange("p (c y) -> p c y", c=4))
                    return
                # ---- G slot (rotation gather)
                g_slot = slot_id
                scr = scr_pool.tile([SCRATCH_ELEMS], dt.bfloat16, name="scr")
                rowmaj = rowmaj_pool.tile([128, 2, W, CPAD], dt.bfloat16,
                                          name="rowmaj")
                scr_m = scr[:].rearrange("(b r sc) -> b r sc", b=128, sc=128)
                scr_v = scr_m.rearrange("b (rt p) sc -> p rt b sc", rt=2)
                for rt in range(2):
                    nc.scalar.copy(
                        out=rowmaj[:, rt, :, 0:C],
                        in_=canon[:, rt, :, :].rearrange("p c x -> p x c"))
                    nc.sync.dma_start(
                        out=scr_v[:, rt, :, 0:64],
                        in_=rowmaj[:, rt, :, :]
                        .rearrange("p (b two) c -> p b (two c)", two=2))
                    nc.sync.dma_start(
                        out=scr_v[:, rt, 0:127, 64:128],
                        in_=rowmaj[:, rt, 2:256, :]
                        .rearrange("p (b two) c -> p b (two c)", two=2))
                    nc.sync.dma_start(
                        out=scr_v[:, rt, 127:128, 64:128],
                        in_=rowmaj[:, rt, 0:2, :]
                        .rearrange("p (b two) c -> p b (two c)", two=2))
                idxt = idx_pool.tile([128, NPIX // 16], dt.int16, name="idxt")
                nc.sync.dma_start(out=idxt[:, :], in_=idx_in[g_slot])
                wgtt = wgt_pool.tile([128, 6, NPIX // 128], dt.bfloat16,
                                     name="wgtt")
                nc.sync.dma_start(out=wgtt[:, :, :], in_=wgt_in[g_slot])
                srcv = scr[:]
                srcv.ap = bass_rust.VecI64Pair([(128, 32767), (1, 256)])
                for ck in range(N_CHUNKS):
                    gbuf = gbuf_pool.tile([128, CHUNK_PIX // 128, 256],
                                          dt.bfloat16, name="gbuf")
                    for g in range(CHUNK_PIX // GSUB):
                        i0 = (CHUNK_PIX // 16) * ck + (GSUB // 16) * g
                        nc.gpsimd.dma_gather(
                            gbuf[:, (GSUB // 128) * g:(GSUB // 128) * (g + 1), :],
                            srcv, idxt[:, i0:i0 + GSUB // 16],
                            num_idxs=GSUB, num_idxs_reg=GSUB,
                            elem_size=256, elem_step=128, single_packet=False,
                            queue_num=g % 4)
                    # materialize the 6 per-pixel weight fields expanded
                    # over channels (unit inner stride for the blends)
                    wexp = wexp_pool.tile([128, 6, CHUNK_COLS, 2, C],
                                          dt.bfloat16, name="wexp")
                    wsrc = wgtt[:, :, (CHUNK_PIX // 128) * ck:
                                (CHUNK_PIX // 128) * (ck + 1)] \
                        .rearrange("p t (x yt) -> p t x yt", yt=2).copy()
                    wsrc.ap = bass_rust.VecI64Pair(list(wsrc.ap) + [(0, C)])
                    nc.scalar.copy(out=wexp[:, :, :, :, :], in_=wsrc)
                    acc = btmp_pool.tile([128, CHUNK_COLS, 2, C],
                                         dt.bfloat16, name="acc")
                    tmp = btmp_pool.tile([128, CHUNK_COLS, 2, C],
                                         dt.bfloat16, name="tmp")
                    co = canon[:, :, :,
                               CHUNK_COLS * ck:CHUNK_COLS * (ck + 1)] \
                        .rearrange("p yt c x -> p x yt c")

                    for ti, (r, sl) in enumerate(
                            [(0, 0), (0, 1), (0, 2), (1, 0), (1, 1), (1, 2)]):
                        eview = gbuf[:, :, 128 * r + 32 * sl:
                                     128 * r + 32 * sl + C] \
                            .rearrange("p (x yt) c -> p x yt c", yt=2)
                        if ti == 0:
                            nc.vector.tensor_tensor(
                                acc[:, :, :, :], eview, wexp[:, ti],
                                Alu.mult)
                        else:
                            nc.vector.tensor_tensor(
                                tmp[:, :, :, :], eview, wexp[:, ti],
                                Alu.mult)
                            nc.vector.tensor_tensor(
                                co if ti == 5 else acc[:, :, :, :],
                                acc[:, :, :, :], tmp[:, :, :, :], Alu.add)

            def emit_store(s):
                canon = canons[s]
                for yt in range(2):
                    for ch in range(4):
                        stage2 = stage_pool.tile([128, 6, W], dt.float32,
                                                 name="stage2")
                        nc.scalar.copy(
                            out=stage2[:, :, :],
                            in_=canon[:, yt, 6 * ch:6 * (ch + 1), :])
                        nc.sync.dma_start(
                            out=out_t[s, 6 * ch:6 * (ch + 1),
                                      128 * yt:128 * (yt + 1), :]
                            .rearrange("c p x -> p c x"),
                            in_=stage2[:, :, :])

            # ---- emission schedule: slot ids in host fill order -----------
            slot_ids = {}
            next_id = {"P": 0, "G": 0}
            for s in range(SAMPLES_PER_CORE):
                for k, ty in enumerate(template[s]):
                    slot_ids[(s, k)] = next_id[ty]
                    next_id[ty] += 1

            # segments end right after each G slot so that interleaving two
            # samples' segments separates their gather groups (the gpsimd
            # engine executes in order; a G slot that depends on its own
            # sample's preceding P slot would otherwise stall the queue).
            def segments(s):
                segs = [[("load", None)]]
                for k, ty in enumerate(template[s]):
                    segs[-1].append((ty, slot_ids[(s, k)]))
                    if ty == "G":
                        segs.append([])
                segs[-1].append(("store", None))
                return [sg for sg in segs if sg]

            for pair in range(0, SAMPLES_PER_CORE, 2):
                queues = [segments(s) for s in
                          (pair, pair + 1) if s < SAMPLES_PER_CORE]
                si = 0
                while any(queues):
                    qi = si % len(queues)
                    si += 1
                    if not queues[qi]:
                        continue
                    seg = queues[qi].pop(0)
                    s = pair + qi
                    for item, sid in seg:
                        if item == "load":
                            emit_load(s)
                        elif item == "store":
                            emit_store(s)
                        else:
                            emit_slot(s, item, sid)

    _split_multi_waits(nc)
    nc.compile()
    return nc


def _split_multi_waits(nc):
    """This container's walrus supports one sync wait per instruction; hoist
    extras onto NoOps."""
    import concourse.mybir as mybir
    n = 0
    for fn in nc.m.functions:
        for bb in fn.blocks:
            out = []
            changed = False
            for inst in bb.instructions:
                si = inst.sync_info
                if si is not None and len(si.on_wait) > 1:
                    waits = list(si.on_wait)
                    for wt in waits[:-1]:
                        nop = mybir.InstNoOp(name=f"wsplit-{n}", ins=[], outs=[])
                        n += 1
                        nop.engine = inst.engine
                        nop.sync_info = mybir.SyncInfo(on_update=[], on_wait=[wt])
                        out.append(nop)
                    si.on_wait = [waits[-1]]
                    changed = True
                out.append(inst)
            if changed:
                bb.instructions = out
    return n


# ---------------------------------------------------------------------------
# entry point

def kernel(images, mask_flip, mask_rot90, mask_trans, mask_scale1, mask_rot1,
           mask_scale2, mask_rot2, scale1, angle1, scale2, angle2,
           rot90_k, shift_h, shift_w):
    import ml_dtypes
    from concourse.bass_utils import run_bass_kernel_spmd

    B = images.shape[0]
    imgs = np.ascontiguousarray(images, dtype=np.float32).reshape(B, C, H, W)
    rot90_k = int(rot90_k)
    shift_h = int(shift_h)
    shift_w = int(shift_w)

    all_slots = []
    for i in range(B):
        slots = _sample_slots(
            bool(mask_scale1[i]), float(scale1[i]),
            bool(mask_rot1[i]), float(angle1[i]),
            bool(mask_scale2[i]), np.asarray(scale2[i], dtype=np.float64),
            bool(mask_rot2[i]), float(angle2[i]))
        all_slots.append(slots)

    seqs = [tuple(ty for ty, _ in sl) for sl in all_slots]
    groups = _optimize_groups(seqs)
    template = tuple(
        _min_template([seqs[i] for i in grp])[0] for grp in groups)

    if template not in _GRAPH_CACHE:
        _GRAPH_CACHE[template] = _build_graph(template)
    nc = _GRAPH_CACHE[template]
    NG = sum(t.count("G") for t in template)
    NP_ = sum(t.count("P") for t in template)

    id_idx, id_w = _identity_g()
    id_vh = _identity_p()
    in_maps = []
    placement = np.zeros((N_CORES, SAMPLES_PER_CORE), np.int64)
    for core in range(N_CORES):
        img_c = np.empty((SAMPLES_PER_CORE, C, H, W), np.float32)
        # (cast to bf16 below, after the permutation stages)
        idx_c = np.empty((max(NG, 1), 128, NPIX // 16), np.int16)
        wgt_c = np.empty((max(NG, 1), 128, 6, NPIX // 128), np.float32)
        vh_c = np.empty((max(NP_, 1), 2, 128, 2, 256), np.float32)
        if NG == 0:
            idx_c[0], wgt_c[0] = id_idx, id_w
        if NP_ == 0:
            vh_c[0] = id_vh
        g_slot = 0
        p_slot = 0
        for p in range(SAMPLES_PER_CORE):
            samp = groups[p][core]
            placement[core, p] = samp
            im = imgs[samp]
            # host-applied permutation stages (pure index permutations)
            if mask_flip[samp]:
                im = im[:, :, ::-1]
            if mask_rot90[samp]:
                im = np.rot90(im, k=rot90_k, axes=(-2, -1))
            if mask_trans[samp]:
                im = np.roll(im, (shift_h, shift_w), axis=(-2, -1))
            img_c[p] = im
            slots = list(all_slots[samp])
            for ty in template[p]:
                if slots and slots[0][0] == ty:
                    payload = slots.pop(0)[1]
                else:
                    payload = None
                if ty == "P":
                    vh_c[p_slot] = id_vh if payload is None else _pe_arrays(*payload)
                    p_slot += 1
                else:
                    if payload is None:
                        idx_c[g_slot], wgt_c[g_slot] = id_idx, id_w
                    else:
                        idx_c[g_slot], wgt_c[g_slot] = _gather_arrays(*payload)
                    g_slot += 1
            assert not slots, (samp, template[p], seqs[samp])
        in_maps.append({
            "img": img_c.astype(ml_dtypes.bfloat16),
            "idx": idx_c,
            "wgt": wgt_c.astype(ml_dtypes.bfloat16),
            "vh": vh_c.astype(ml_dtypes.bfloat16),
        })

    trace = os.environ.get("KTRACE", "0") == "1"
    res = run_bass_kernel_spmd(nc, in_maps, core_ids=list(range(N_CORES)),
                               trace=trace)
    kernel._last_result = res

    out = np.empty((B, C, H, W), np.float32)
    for core in range(N_CORES):
        o = np.asarray(res.results[core]["out"], dtype=np.float32)
        for p in range(SAMPLES_PER_CORE):
            out[placement[core, p]] = o[p]
    return out.reshape(B, 3, 8, H, W)


# revision 19
# speedup vs baseline: 1.0619x; 1.0619x over previous
"""Trainium2 (8 NeuronCores, Bass/Tile) kernel for
nn_AdaptiveDiscriminatorAugmentation.

Strategy
--------
Pure data parallel: 32 samples -> 8 cores x 4 sample positions, assigned by a
grouping optimizer so per-position warp templates minimize total slot cost.
The integer permutation stages (flip / rot90 / roll) are applied while
assembling the per-core input shards (pure index permutations, part of the
same host copy that builds the shards).  The resampling stages run on device
as a uniform SPMD program; all per-sample variation (which scale/angle,
identity padding) is carried as data:

- separable warps (scale1 / scale2 / identity padding) = two TensorEngine
  one-hot matmul passes with host-built bf16 hat matrices:
      pass1: T1[c, y] = sum_r I[r, c] * V[r, y]   (stationary = image tile)
      pass2: out[y, x] = sum_c T1[c, y] * H[c, x] (stationary = T1 tile)
  no transposes needed: pass1 emits column-major, pass2 restores row-major.
  PSUM->SBUF copies are merged 4 channels at a time and alternate between
  the Scalar and Vector engines.
- rotation warps = per-pixel bilinear gather:
      relayout (Scalar) -> blocked HBM scratch write (DMA) -> dma_gather
      (512B elems = 2 rows x 4px x 32ch bf16, int16 idx = (c0>>1)*256+r0,
      4 gathers/chunk round-robined over 4 SWDGE queues so descriptor
      generation parallelizes) -> 11-op DVE blend with 6 weight fields
      (x-parity folded into the slot weights) materialized per chunk on the
      Scalar engine so every blend operand has unit inner stride.
  dma_gather is limited to 1024 indices per call (SWDGE ring capacity).

Per-position slot templates are minimal-cost subsequences of [P, G, P, G]
(= scale1, rot1, scale2, rot2); a local-search optimizer assigns samples to
positions to minimize total gather (G) and matmul (P) slots; padding slots
run exact identities (bf16-exact hat/weight values).
"""
import math
import os
import sys

import numpy as np

sys.path.insert(0, "/opt/trn_rl_repo")

H = W = 256
C = 24
CPAD = 32
NPIX = H * W
CHUNK_COLS = 16
CHUNK_PIX = CHUNK_COLS * H
N_CHUNKS = W // CHUNK_COLS
N_CORES = 8
SAMPLES_PER_CORE = 4
SCRATCH_ELEMS = 128 * 256 * 128
GSUB = 512                       # dma_gather calls sized so two fit per SWDGE queue ring

_xs = np.arange(W)
I_OF_YX = ((_xs // CHUNK_COLS)[None, :] * CHUNK_PIX
           + (_xs % CHUNK_COLS)[None, :] * H + np.arange(H)[:, None])


# ---------------------------------------------------------------------------
# host-side warp math

def _warp_coords(angle_deg, sy, sx):
    cy = cx = 0.5 * H
    th = angle_deg * (math.pi / 180.0)
    cos, sin = np.cos(th), np.sin(th)
    ys, xs = np.meshgrid(np.arange(H, dtype=np.float64),
                         np.arange(W, dtype=np.float64), indexing="ij")
    dy, dx = ys - cy, xs - cx
    xin = (cos * dx + sin * dy) / sx + cx
    yin = (-sin * dx + cos * dy) / sy + cy
    m = float(H - 1)

    def reflect(c):
        c = np.abs(c) % (2.0 * m)
        return np.minimum(c, 2.0 * m - c)

    return reflect(yin), reflect(xin)


def _axis_coords(scale):
    """1D inverse-map + reflect for a separable (axis-aligned) warp."""
    cy = 0.5 * H
    v = (np.arange(H, dtype=np.float64) - cy) / scale + cy
    m = float(H - 1)
    v = np.abs(v) % (2.0 * m)
    return np.minimum(v, 2.0 * m - v)


def _hat_matrix(v):
    """[256 src, 256 dst] bilinear hat weights for dst coords v (float64)."""
    r0 = np.minimum(np.floor(v), 254.0).astype(np.int64)
    w = (v - r0).astype(np.float32)
    M = np.zeros((H, H), np.float32)
    j = np.arange(H)
    M[r0, j] += 1.0 - w
    M[r0 + 1, j] += w
    return M


def _sample_slots(m_s1, s1, m_r1, a1, m_s2, s2, m_r2, a2):
    """Slot list for one sample: ("P", (vy, vx)) or ("G", (yin, xin)).
    Only the stages the sample actually needs (order: s1, r1, s2, r2)."""
    slots = []
    if m_s1:
        slots.append(("P", (_axis_coords(s1), _axis_coords(s1))))
    if m_r1:
        slots.append(("G", _warp_coords(a1, 1.0, 1.0)))
    if m_s2:
        slots.append(("P", (_axis_coords(s2[0]), _axis_coords(s2[1]))))
    if m_r2:
        slots.append(("G", _warp_coords(a2, 1.0, 1.0)))
    return slots


def _gather_arrays(yin, xin):
    """(idx [128, NPIX//16] int16, W [128, 6, NPIX//128] float32)."""
    ii = np.arange(NPIX)
    yin_f = np.empty(NPIX)
    xin_f = np.empty(NPIX)
    yin_f[I_OF_YX.ravel()] = yin.ravel()
    xin_f[I_OF_YX.ravel()] = xin.ravel()
    r0 = np.minimum(np.floor(yin_f), 254.0)
    c0 = np.minimum(np.floor(xin_f), 254.0)
    wy = (yin_f - r0).astype(np.float32)
    wx = (xin_f - c0).astype(np.float32)
    par = (c0.astype(np.int64) & 1).astype(np.float32)
    idx = (c0.astype(np.int64) >> 1) * 256 + r0.astype(np.int64)
    assert 0 <= idx.min() and idx.max() <= 32766
    idx_w = np.zeros((16, NPIX // 16), np.int16)
    idx_w[ii % 16, ii // 16] = idx.astype(np.int16)
    idx_wrapped = np.tile(idx_w, (8, 1))
    s0 = (1 - par) * (1 - wx)
    s1 = par * (1 - wx) + (1 - par) * wx
    s2 = par * wx
    v0, v1 = (1 - wy), wy
    Wf = np.stack([v0 * s0, v0 * s1, v0 * s2, v1 * s0, v1 * s1, v1 * s2])
    Wdev = np.zeros((128, 6, NPIX // 128), np.float32)
    Wdev[ii % 128, :, ii // 128] = Wf.astype(np.float32).T
    return idx_wrapped, Wdev


def _pe_arrays(vy, vx):
    """vh [2, 128, 2, 256] float32: [0] = V[r, y], [1] = H[c, x];
    partition-major p = src & 127, rt = src >> 7."""
    V = _hat_matrix(vy)
    Hm = _hat_matrix(vx)
    out = np.zeros((2, 128, 2, 256), np.float32)
    src = np.arange(H)
    out[0, src & 127, src >> 7, :] = V
    out[1, src & 127, src >> 7, :] = Hm
    return out


_ID_G = None
_ID_P = None


def _identity_g():
    global _ID_G
    if _ID_G is None:
        ys, xs = np.meshgrid(np.arange(H, dtype=np.float64),
                             np.arange(W, dtype=np.float64), indexing="ij")
        _ID_G = _gather_arrays(ys, xs)
    return _ID_G


def _identity_p():
    global _ID_P
    if _ID_P is None:
        ident = np.arange(H, dtype=np.float64)
        _ID_P = _pe_arrays(ident, ident)
    return _ID_P


# ---------------------------------------------------------------------------
# grouping optimizer: assign samples to the 4 positions so the merged
# per-position templates minimize total slot cost.

_COST_G = 240
_COST_P = 100
_TPOS = ("P", "G", "P", "G")


def _embeds(seq, t):
    """Greedy subsequence embedding of a sample slot-type seq into template t."""
    j = 0
    for ty in t:
        if j < len(seq) and seq[j] == ty:
            j += 1
    return j == len(seq)


def _min_template(seqs):
    best, bc = None, None
    for mask in range(16):
        t = tuple(_TPOS[k] for k in range(4) if (mask >> k) & 1)
        if all(_embeds(s, t) for s in seqs):
            c = sum(_COST_G if ty == "G" else _COST_P for ty in t)
            if bc is None or c < bc:
                best, bc = t, c
    assert best is not None
    return best, bc


def _optimize_groups(seqs_all):
    B = len(seqs_all)
    order = sorted(range(B),
                   key=lambda i: (seqs_all[i].count("G"), seqs_all[i]),
                   reverse=True)
    groups = [order[p * N_CORES:(p + 1) * N_CORES]
              for p in range(SAMPLES_PER_CORE)]

    def gcost(g):
        return _min_template([seqs_all[i] for i in g])[1]

    costs = [gcost(g) for g in groups]
    improved = True
    while improved:
        improved = False
        for a in range(SAMPLES_PER_CORE):
            for b in range(a + 1, SAMPLES_PER_CORE):
                for ia in range(N_CORES):
                    for ib in range(N_CORES):
                        ga = groups[a][:]
                        gb = groups[b][:]
                        ga[ia], gb[ib] = gb[ib], ga[ia]
                        ca, cb = gcost(ga), gcost(gb)
                        if ca + cb < costs[a] + costs[b]:
                            groups[a], groups[b] = ga, gb
                            costs[a], costs[b] = ca, cb
                            improved = True
    # heaviest positions first so their gathers start early
    order_p = sorted(range(SAMPLES_PER_CORE), key=lambda p: -costs[p])
    groups = [groups[p] for p in order_p]
    return groups


# ---------------------------------------------------------------------------
# device program

_GRAPH_CACHE = {}


def _build_graph(template):
    """template: tuple per position of slot-type strings, e.g.
    (('P','G','P','G'), ('P','G'), ('P',), ('P',))."""
    import concourse.bacc as bacc
    import concourse.mybir as mybir
    import bass_rust
    from concourse.tile import TileContext
    from concourse.library_config import mlp

    dt = mybir.dt
    Alu = mybir.AluOpType
    NG = sum(t.count("G") for t in template)
    NP_ = sum(t.count("P") for t in template)

    nc = bacc.Bacc("TRN2", num_swdge_queues=4)
    img_in = nc.declare_dram_parameter(
        "img", [SAMPLES_PER_CORE, C, H, W], dt.float32, isOutput=False)
    idx_in = nc.declare_dram_parameter(
        "idx", [max(NG, 1), 128, NPIX // 16], dt.int16, isOutput=False)
    wgt_in = nc.declare_dram_parameter(
        "wgt", [max(NG, 1), 128, 6, NPIX // 128], dt.bfloat16, isOutput=False)
    vh_in = nc.declare_dram_parameter(
        "vh", [max(NP_, 1), 2, 128, 2, 256], dt.bfloat16, isOutput=False)
    out_t = nc.declare_dram_parameter(
        "out", [SAMPLES_PER_CORE, C, H, W], dt.float32, isOutput=True)

    copy_tick = 0

    with TileContext(nc) as tc:
        nc.gpsimd.load_library(mlp)
        with (tc.tile_pool(name="scrp", bufs=2, space="DRAM") as scr_pool,
              tc.tile_pool(name="psum", bufs=4, space="PSUM") as psum_pool,
              tc.tile_pool(name="canon", bufs=2) as canon_pool,
              tc.tile_pool(name="stage", bufs=2) as stage_pool,
              tc.tile_pool(name="rowmaj", bufs=1) as rowmaj_pool,
              tc.tile_pool(name="gbuf", bufs=2) as gbuf_pool,
              tc.tile_pool(name="idxp", bufs=1) as idx_pool,
              tc.tile_pool(name="wgtp", bufs=2) as wgt_pool,
              tc.tile_pool(name="wexp", bufs=2) as wexp_pool,
              tc.tile_pool(name="t1p", bufs=1) as t1_pool,
              tc.tile_pool(name="vhp", bufs=2) as vh_pool,
              tc.tile_pool(name="btmp", bufs=1) as btmp_pool):

            def merged_copy(out_ap, ps_ap):
                """PSUM->SBUF copy, alternating Scalar / Vector."""
                nonlocal copy_tick
                if copy_tick % 2 == 0:
                    nc.scalar.copy(out=out_ap, in_=ps_ap)
                else:
                    nc.vector.tensor_copy(out_ap, ps_ap)
                copy_tick += 1

            canons = {}

            def emit_load(s):
                canon = canon_pool.tile([128, 2, C, W], dt.bfloat16,
                                        name="canon")
                canons[s] = canon
                # ---- load + cast to bf16 canonical [p=y&127, yt, c, x]
                for yt in range(2):
                    for ch in range(4):
                        stage = stage_pool.tile([128, 6, W], dt.float32,
                                                name="stage")
                        nc.sync.dma_start(
                            out=stage[:, :, :],
                            in_=img_in[s, 6 * ch:6 * (ch + 1),
                                       128 * yt:128 * (yt + 1), :]
                            .rearrange("c p x -> p c x"))
                        nc.scalar.copy(
                            out=canon[:, yt, 6 * ch:6 * (ch + 1), :],
                            in_=stage[:, :, :])

            def emit_slot(s, ty, slot_id):
                canon = canons[s]
                if ty == "P":
                    p_slot = slot_id
                    vh = vh_pool.tile([128, 2, 2, 256], dt.bfloat16,
                                      name="vh")
                    nc.sync.dma_start(
                        out=vh[:, :, :, :],
                        in_=vh_in[p_slot].rearrange("w p rt f -> p w rt f"))
                    t1 = t1_pool.tile([128, 2, C, 256], dt.bfloat16,
                                      name="t1")
                    # pass 1: T1[c, y] = sum_r I[r, c] V[r, y]
                    for ct in range(2):
                        for q in range(C // 4):
                            ps = psum_pool.tile([128, 1024], dt.float32,
                                                name="ps")
                            for k in range(4):
                                ch = 4 * q + k
                                for rt in range(2):
                                    nc.tensor.matmul(
                                        ps[:, 256 * k:256 * (k + 1)],
                                        canon[:, rt, ch,
                                              128 * ct:128 * (ct + 1)],
                                        vh[:, 0, rt, :],
                                        start=(rt == 0), stop=(rt == 1))
                            merged_copy(
                                t1[:, ct, 4 * q:4 * (q + 1), :],
                                ps[:, :].rearrange("p (c y) -> p c y", c=4))
                    # pass 2: out[y, x] = sum_c T1[c, y] H[c, x]
                    for yt in range(2):
                        for q in range(C // 4):
                            ps2 = psum_pool.tile([128, 1024], dt.float32,
                                                 name="ps")
                            for k in range(4):
                                ch = 4 * q + k
                                for ct in range(2):
                                    nc.tensor.matmul(
                                        ps2[:, 256 * k:256 * (k + 1)],
                                        t1[:, ct, ch,
                                           128 * yt:128 * (yt + 1)],
                                        vh[:, 1, ct, :],
                                        start=(ct == 0), stop=(ct == 1))
                            merged_copy(
                                canon[:, yt, 4 * q:4 * (q + 1), :],
                                ps2[:, :].rearrange("p (c y) -> p c y", c=4))
                    return
                # ---- G slot (rotation gather)
                g_slot = slot_id
                scr = scr_pool.tile([SCRATCH_ELEMS], dt.bfloat16, name="scr")
                rowmaj = rowmaj_pool.tile([128, 2, W, CPAD], dt.bfloat16,
                                          name="rowmaj")
                scr_m = scr[:].rearrange("(b r sc) -> b r sc", b=128, sc=128)
                scr_v = scr_m.rearrange("b (rt p) sc -> p rt b sc", rt=2)
                for rt in range(2):
                    nc.scalar.copy(
                        out=rowmaj[:, rt, :, 0:C],
                        in_=canon[:, rt, :, :].rearrange("p c x -> p x c"))
                    nc.sync.dma_start(
                        out=scr_v[:, rt, :, 0:64],
                        in_=rowmaj[:, rt, :, :]
                        .rearrange("p (b two) c -> p b (two c)", two=2))
                    nc.sync.dma_start(
                        out=scr_v[:, rt, 0:127, 64:128],
                        in_=rowmaj[:, rt, 2:256, :]
                        .rearrange("p (b two) c -> p b (two c)", two=2))
                    nc.sync.dma_start(
                        out=scr_v[:, rt, 127:128, 64:128],
                        in_=rowmaj[:, rt, 0:2, :]
                        .rearrange("p (b two) c -> p b (two c)", two=2))
                idxt = idx_pool.tile([128, NPIX // 16], dt.int16, name="idxt")
                nc.sync.dma_start(out=idxt[:, :], in_=idx_in[g_slot])
                wgtt = wgt_pool.tile([128, 6, NPIX // 128], dt.bfloat16,
                                     name="wgtt")
                nc.sync.dma_start(out=wgtt[:, :, :], in_=wgt_in[g_slot])
                srcv = scr[:]
                srcv.ap = bass_rust.VecI64Pair([(128, 32767), (1, 256)])
                for ck in range(N_CHUNKS):
                    gbuf = gbuf_pool.tile([128, CHUNK_PIX // 128, 256],
                                          dt.bfloat16, name="gbuf")
                    for g in range(CHUNK_PIX // GSUB):
                        i0 = (CHUNK_PIX // 16) * ck + (GSUB // 16) * g
                        nc.gpsimd.dma_gather(
                            gbuf[:, (GSUB // 128) * g:(GSUB // 128) * (g + 1), :],
                            srcv, idxt[:, i0:i0 + GSUB // 16],
                            num_idxs=GSUB, num_idxs_reg=GSUB,
                            elem_size=256, elem_step=128, single_packet=False,
                            queue_num=g % 4)
                    # materialize the 6 per-pixel weight fields expanded
                    # over channels (unit inner stride for the blends)
                    wexp = wexp_pool.tile([128, 6, CHUNK_COLS, 2, C],
                                          dt.bfloat16, name="wexp")
                    wsrc = wgtt[:, :, (CHUNK_PIX // 128) * ck:
                                (CHUNK_PIX // 128) * (ck + 1)] \
                        .rearrange("p t (x yt) -> p t x yt", yt=2).copy()
                    wsrc.ap = bass_rust.VecI64Pair(list(wsrc.ap) + [(0, C)])
                    nc.scalar.copy(out=wexp[:, :, :, :, :], in_=wsrc)
                    acc = btmp_pool.tile([128, CHUNK_COLS, 2, C],
                                         dt.bfloat16, name="acc")
                    tmp = btmp_pool.tile([128, CHUNK_COLS, 2, C],
                                         dt.bfloat16, name="tmp")
                    co = canon[:, :, :,
                               CHUNK_COLS * ck:CHUNK_COLS * (ck + 1)] \
                        .rearrange("p yt c x -> p x yt c")

                    for ti, (r, sl) in enumerate(
                            [(0, 0), (0, 1), (0, 2), (1, 0), (1, 1), (1, 2)]):
                        eview = gbuf[:, :, 128 * r + 32 * sl:
                                     128 * r + 32 * sl + C] \
                            .rearrange("p (x yt) c -> p x yt c", yt=2)
                        if ti == 0:
                            nc.vector.tensor_tensor(
                                acc[:, :, :, :], eview, wexp[:, ti],
                                Alu.mult)
                        else:
                            nc.vector.tensor_tensor(
                                tmp[:, :, :, :], eview, wexp[:, ti],
                                Alu.mult)
                            nc.vector.tensor_tensor(
                                co if ti == 5 else acc[:, :, :, :],
                                acc[:, :, :, :], tmp[:, :, :, :], Alu.add)

            def emit_store(s):
                canon = canons[s]
                for yt in range(2):
                    for ch in range(4):
                        stage2 = stage_pool.tile([128, 6, W], dt.float32,
                                                 name="stage2")
                        nc.scalar.copy(
                            out=stage2[:, :, :],
                            in_=canon[:, yt, 6 * ch:6 * (ch + 1), :])
                        nc.sync.dma_start(
                            out=out_t[s, 6 * ch:6 * (ch + 1),
                                      128 * yt:128 * (yt + 1), :]
                            .rearrange("c p x -> p c x"),
                            in_=stage2[:, :, :])

            # ---- emission schedule: slot ids in host fill order -----------
            slot_ids = {}
            next_id = {"P": 0, "G": 0}
            for s in range(SAMPLES_PER_CORE):
                for k, ty in enumerate(template[s]):
                    slot_ids[(s, k)] = next_id[ty]
                    next_id[ty] += 1

            # segments end right after each G slot so that interleaving two
            # samples' segments separates their gather groups (the gpsimd
            # engine executes in order; a G slot that depends on its own
            # sample's preceding P slot would otherwise stall the queue).
            def segments(s):
                segs = [[("load", None)]]
                for k, ty in enumerate(template[s]):
                    segs[-1].append((ty, slot_ids[(s, k)]))
                    if ty == "G":
                        segs.append([])
                segs[-1].append(("store", None))
                return [sg for sg in segs if sg]

            for pair in range(0, SAMPLES_PER_CORE, 2):
                queues = [segments(s) for s in
                          (pair, pair + 1) if s < SAMPLES_PER_CORE]
                si = 0
                while any(queues):
                    qi = si % len(queues)
                    si += 1
                    if not queues[qi]:
                        continue
                    seg = queues[qi].pop(0)
                    s = pair + qi
                    for item, sid in seg:
                        if item == "load":
                            emit_load(s)
                        elif item == "store":
                            emit_store(s)
                        else:
                            emit_slot(s, item, sid)

    _split_multi_waits(nc)
    nc.compile()
    return nc


def _split_multi_waits(nc):
    """This container's walrus supports one sync wait per instruction; hoist
    extras onto NoOps."""
    import concourse.mybir as mybir
    n = 0
    for fn in nc.m.functions:
        for bb in fn.blocks:
            out = []
            changed = False
            for inst in bb.instructions:
                si = inst.sync_info
                if si is not None and len(si.on_wait) > 1:
                    waits = list(si.on_wait)
                    for wt in waits[:-1]:
                        nop = mybir.InstNoOp(name=f"wsplit-{n}", ins=[], outs=[])
                        n += 1
                        nop.engine = inst.engine
                        nop.sync_info = mybir.SyncInfo(on_update=[], on_wait=[wt])
                        out.append(nop)
                    si.on_wait = [waits[-1]]
                    changed = True
                out.append(inst)
            if changed:
                bb.instructions = out
    return n


# ---------------------------------------------------------------------------
# entry point

def kernel(images, mask_flip, mask_rot90, mask_trans, mask_scale1, mask_rot1,
           mask_scale2, mask_rot2, scale1, angle1, scale2, angle2,
           rot90_k, shift_h, shift_w):
    import ml_dtypes
    from concourse.bass_utils import run_bass_kernel_spmd

    B = images.shape[0]
    imgs = np.ascontiguousarray(images, dtype=np.float32).reshape(B, C, H, W)
    rot90_k = int(rot90_k)
    shift_h = int(shift_h)
    shift_w = int(shift_w)

    all_slots = []
    for i in range(B):
        slots = _sample_slots(
            bool(mask_scale1[i]), float(scale1[i]),
            bool(mask_rot1[i]), float(angle1[i]),
            bool(mask_scale2[i]), np.asarray(scale2[i], dtype=np.float64),
            bool(mask_rot2[i]), float(angle2[i]))
        all_slots.append(slots)

    seqs = [tuple(ty for ty, _ in sl) for sl in all_slots]
    groups = _optimize_groups(seqs)
    template = tuple(
        _min_template([seqs[i] for i in grp])[0] for grp in groups)

    if template not in _GRAPH_CACHE:
        _GRAPH_CACHE[template] = _build_graph(template)
    nc = _GRAPH_CACHE[template]
    NG = sum(t.count("G") for t in template)
    NP_ = sum(t.count("P") for t in template)

    id_idx, id_w = _identity_g()
    id_vh = _identity_p()
    in_maps = []
    placement = np.zeros((N_CORES, SAMPLES_PER_CORE), np.int64)
    for core in range(N_CORES):
        img_c = np.empty((SAMPLES_PER_CORE, C, H, W), np.float32)
        idx_c = np.empty((max(NG, 1), 128, NPIX // 16), np.int16)
        wgt_c = np.empty((max(NG, 1), 128, 6, NPIX // 128), np.float32)
        vh_c = np.empty((max(NP_, 1), 2, 128, 2, 256), np.float32)
        if NG == 0:
            idx_c[0], wgt_c[0] = id_idx, id_w
        if NP_ == 0:
            vh_c[0] = id_vh
        g_slot = 0
        p_slot = 0
        for p in range(SAMPLES_PER_CORE):
            samp = groups[p][core]
            placement[core, p] = samp
            im = imgs[samp]
            # host-applied permutation stages (pure index permutations)
            if mask_flip[samp]:
                im = im[:, :, ::-1]
            if mask_rot90[samp]:
                im = np.rot90(im, k=rot90_k, axes=(-2, -1))
            if mask_trans[samp]:
                im = np.roll(im, (shift_h, shift_w), axis=(-2, -1))
            img_c[p] = im
            slots = list(all_slots[samp])
            for ty in template[p]:
                if slots and slots[0][0] == ty:
                    payload = slots.pop(0)[1]
                else:
                    payload = None
                if ty == "P":
                    vh_c[p_slot] = id_vh if payload is None else _pe_arrays(*payload)
                    p_slot += 1
                else:
                    if payload is None:
                        idx_c[g_slot], wgt_c[g_slot] = id_idx, id_w
                    else:
                        idx_c[g_slot], wgt_c[g_slot] = _gather_arrays(*payload)
                    g_slot += 1
            assert not slots, (samp, template[p], seqs[samp])
        in_maps.append({
            "img": img_c,
            "idx": idx_c,
            "wgt": wgt_c.astype(ml_dtypes.bfloat16),
            "vh": vh_c.astype(ml_dtypes.bfloat16),
        })

    trace = os.environ.get("KTRACE", "0") == "1"
    res = run_bass_kernel_spmd(nc, in_maps, core_ids=list(range(N_CORES)),
                               trace=trace)
    kernel._last_result = res

    out = np.empty((B, C, H, W), np.float32)
    for core in range(N_CORES):
        o = np.asarray(res.results[core]["out"], dtype=np.float32)
        for p in range(SAMPLES_PER_CORE):
            out[placement[core, p]] = o[p]
    return out.reshape(B, 3, 8, H, W)


# revision 20
# speedup vs baseline: 1.0714x; 1.0090x over previous
"""Trainium2 (8 NeuronCores, Bass/Tile) kernel for
nn_AdaptiveDiscriminatorAugmentation.

Strategy
--------
Pure data parallel: 32 samples -> 8 cores x 4 sample positions, assigned by a
grouping optimizer so per-position warp templates minimize total slot cost.
The integer permutation stages (flip / rot90 / roll) are applied while
assembling the per-core input shards (pure index permutations, part of the
same host copy that builds the shards).  The resampling stages run on device
as a uniform SPMD program; all per-sample variation (which scale/angle,
identity padding) is carried as data:

- separable warps (scale1 / scale2 / identity padding) = two TensorEngine
  one-hot matmul passes with host-built bf16 hat matrices:
      pass1: T1[c, y] = sum_r I[r, c] * V[r, y]   (stationary = image tile)
      pass2: out[y, x] = sum_c T1[c, y] * H[c, x] (stationary = T1 tile)
  no transposes needed: pass1 emits column-major, pass2 restores row-major.
  PSUM->SBUF copies are merged 4 channels at a time and alternate between
  the Scalar and Vector engines.
- rotation warps = per-pixel bilinear gather:
      relayout (Scalar) -> blocked HBM scratch write (DMA) -> dma_gather
      (512B elems = 2 rows x 4px x 32ch bf16, int16 idx = (c0>>1)*256+r0,
      4 gathers/chunk round-robined over 4 SWDGE queues so descriptor
      generation parallelizes) -> 11-op DVE blend with 6 weight fields
      (x-parity folded into the slot weights) materialized per chunk on the
      Scalar engine so every blend operand has unit inner stride.
  dma_gather is limited to 1024 indices per call (SWDGE ring capacity).

Per-position slot templates are minimal-cost subsequences of [P, G, P, G]
(= scale1, rot1, scale2, rot2); a local-search optimizer assigns samples to
positions to minimize total gather (G) and matmul (P) slots; padding slots
run exact identities (bf16-exact hat/weight values).
"""
import math
import os
import sys

import numpy as np

sys.path.insert(0, "/opt/trn_rl_repo")

H = W = 256
C = 24
CPAD = 32
NPIX = H * W
CHUNK_COLS = 16
CHUNK_PIX = CHUNK_COLS * H
N_CHUNKS = W // CHUNK_COLS
N_CORES = 8
SAMPLES_PER_CORE = 4
SCRATCH_ELEMS = 128 * 256 * 128
GSUB = 512                       # dma_gather calls sized so two fit per SWDGE queue ring

_xs = np.arange(W)
I_OF_YX = ((_xs // CHUNK_COLS)[None, :] * CHUNK_PIX
           + (_xs % CHUNK_COLS)[None, :] * H + np.arange(H)[:, None])


# ---------------------------------------------------------------------------
# host-side warp math

def _warp_coords(angle_deg, sy, sx):
    cy = cx = 0.5 * H
    th = angle_deg * (math.pi / 180.0)
    cos, sin = np.cos(th), np.sin(th)
    ys, xs = np.meshgrid(np.arange(H, dtype=np.float64),
                         np.arange(W, dtype=np.float64), indexing="ij")
    dy, dx = ys - cy, xs - cx
    xin = (cos * dx + sin * dy) / sx + cx
    yin = (-sin * dx + cos * dy) / sy + cy
    m = float(H - 1)

    def reflect(c):
        c = np.abs(c) % (2.0 * m)
        return np.minimum(c, 2.0 * m - c)

    return reflect(yin), reflect(xin)


def _axis_coords(scale):
    """1D inverse-map + reflect for a separable (axis-aligned) warp."""
    cy = 0.5 * H
    v = (np.arange(H, dtype=np.float64) - cy) / scale + cy
    m = float(H - 1)
    v = np.abs(v) % (2.0 * m)
    return np.minimum(v, 2.0 * m - v)


def _hat_matrix(v):
    """[256 src, 256 dst] bilinear hat weights for dst coords v (float64)."""
    r0 = np.minimum(np.floor(v), 254.0).astype(np.int64)
    w = (v - r0).astype(np.float32)
    M = np.zeros((H, H), np.float32)
    j = np.arange(H)
    M[r0, j] += 1.0 - w
    M[r0 + 1, j] += w
    return M


def _sample_slots(m_s1, s1, m_r1, a1, m_s2, s2, m_r2, a2):
    """Slot list for one sample: ("P", (vy, vx)) or ("G", (yin, xin)).
    Only the stages the sample actually needs (order: s1, r1, s2, r2)."""
    slots = []
    if m_s1:
        slots.append(("P", (_axis_coords(s1), _axis_coords(s1))))
    if m_r1:
        slots.append(("G", _warp_coords(a1, 1.0, 1.0)))
    if m_s2:
        slots.append(("P", (_axis_coords(s2[0]), _axis_coords(s2[1]))))
    if m_r2:
        slots.append(("G", _warp_coords(a2, 1.0, 1.0)))
    return slots


def _gather_arrays(yin, xin):
    """(idx [128, NPIX//16] int16, W [128, 6, NPIX//128] float32)."""
    ii = np.arange(NPIX)
    yin_f = np.empty(NPIX)
    xin_f = np.empty(NPIX)
    yin_f[I_OF_YX.ravel()] = yin.ravel()
    xin_f[I_OF_YX.ravel()] = xin.ravel()
    r0 = np.minimum(np.floor(yin_f), 254.0)
    c0 = np.minimum(np.floor(xin_f), 254.0)
    wy = (yin_f - r0).astype(np.float32)
    wx = (xin_f - c0).astype(np.float32)
    par = (c0.astype(np.int64) & 1).astype(np.float32)
    idx = (c0.astype(np.int64) >> 1) * 256 + r0.astype(np.int64)
    assert 0 <= idx.min() and idx.max() <= 32766
    idx_w = np.zeros((16, NPIX // 16), np.int16)
    idx_w[ii % 16, ii // 16] = idx.astype(np.int16)
    idx_wrapped = np.tile(idx_w, (8, 1))
    s0 = (1 - par) * (1 - wx)
    s1 = par * (1 - wx) + (1 - par) * wx
    s2 = par * wx
    v0, v1 = (1 - wy), wy
    Wf = np.stack([v0 * s0, v0 * s1, v0 * s2, v1 * s0, v1 * s1, v1 * s2])
    Wdev = np.zeros((128, 6, NPIX // 128), np.float32)
    Wdev[ii % 128, :, ii // 128] = Wf.astype(np.float32).T
    return idx_wrapped, Wdev


def _pe_arrays(vy, vx):
    """vh [2, 128, 2, 256] float32: [0] = V[r, y], [1] = H[c, x];
    partition-major p = src & 127, rt = src >> 7."""
    V = _hat_matrix(vy)
    Hm = _hat_matrix(vx)
    out = np.zeros((2, 128, 2, 256), np.float32)
    src = np.arange(H)
    out[0, src & 127, src >> 7, :] = V
    out[1, src & 127, src >> 7, :] = Hm
    return out


_ID_G = None
_ID_P = None


def _identity_g():
    global _ID_G
    if _ID_G is None:
        ys, xs = np.meshgrid(np.arange(H, dtype=np.float64),
                             np.arange(W, dtype=np.float64), indexing="ij")
        _ID_G = _gather_arrays(ys, xs)
    return _ID_G


def _identity_p():
    global _ID_P
    if _ID_P is None:
        ident = np.arange(H, dtype=np.float64)
        _ID_P = _pe_arrays(ident, ident)
    return _ID_P


# ---------------------------------------------------------------------------
# grouping optimizer: assign samples to the 4 positions so the merged
# per-position templates minimize total slot cost.

_COST_G = 240
_COST_P = 100
_TPOS = ("P", "G", "P", "G")


def _embeds(seq, t):
    """Greedy subsequence embedding of a sample slot-type seq into template t."""
    j = 0
    for ty in t:
        if j < len(seq) and seq[j] == ty:
            j += 1
    return j == len(seq)


def _min_template(seqs):
    best, bc = None, None
    for mask in range(16):
        t = tuple(_TPOS[k] for k in range(4) if (mask >> k) & 1)
        if all(_embeds(s, t) for s in seqs):
            c = sum(_COST_G if ty == "G" else _COST_P for ty in t)
            if bc is None or c < bc:
                best, bc = t, c
    assert best is not None
    return best, bc


def _optimize_groups(seqs_all):
    B = len(seqs_all)
    order = sorted(range(B),
                   key=lambda i: (seqs_all[i].count("G"), seqs_all[i]),
                   reverse=True)
    groups = [order[p * N_CORES:(p + 1) * N_CORES]
              for p in range(SAMPLES_PER_CORE)]

    def gcost(g):
        return _min_template([seqs_all[i] for i in g])[1]

    costs = [gcost(g) for g in groups]
    improved = True
    while improved:
        improved = False
        for a in range(SAMPLES_PER_CORE):
            for b in range(a + 1, SAMPLES_PER_CORE):
                for ia in range(N_CORES):
                    for ib in range(N_CORES):
                        ga = groups[a][:]
                        gb = groups[b][:]
                        ga[ia], gb[ib] = gb[ib], ga[ia]
                        ca, cb = gcost(ga), gcost(gb)
                        if ca + cb < costs[a] + costs[b]:
                            groups[a], groups[b] = ga, gb
                            costs[a], costs[b] = ca, cb
                            improved = True
    # heaviest positions first so their gathers start early
    order_p = sorted(range(SAMPLES_PER_CORE), key=lambda p: -costs[p])
    groups = [groups[p] for p in order_p]
    return groups


# ---------------------------------------------------------------------------
# device program

_GRAPH_CACHE = {}


def _build_graph(template):
    """template: tuple per position of slot-type strings, e.g.
    (('P','G','P','G'), ('P','G'), ('P',), ('P',))."""
    import concourse.bacc as bacc
    import concourse.mybir as mybir
    import bass_rust
    from concourse.tile import TileContext
    from concourse.library_config import mlp

    dt = mybir.dt
    Alu = mybir.AluOpType
    NG = sum(t.count("G") for t in template)
    NP_ = sum(t.count("P") for t in template)

    nc = bacc.Bacc("TRN2", num_swdge_queues=4)
    img_in = nc.declare_dram_parameter(
        "img", [SAMPLES_PER_CORE, C, H, W], dt.float32, isOutput=False)
    idx_in = nc.declare_dram_parameter(
        "idx", [max(NG, 1), 128, NPIX // 16], dt.int16, isOutput=False)
    wgt_in = nc.declare_dram_parameter(
        "wgt", [max(NG, 1), 128, 6, NPIX // 128], dt.bfloat16, isOutput=False)
    vh_in = nc.declare_dram_parameter(
        "vh", [max(NP_, 1), 2, 128, 2, 256], dt.bfloat16, isOutput=False)
    out_t = nc.declare_dram_parameter(
        "out", [SAMPLES_PER_CORE, C, H, W], dt.float32, isOutput=True)

    copy_tick = 0

    with TileContext(nc) as tc:
        nc.gpsimd.load_library(mlp)
        with (tc.tile_pool(name="scrp", bufs=2, space="DRAM") as scr_pool,
              tc.tile_pool(name="psum", bufs=4, space="PSUM") as psum_pool,
              tc.tile_pool(name="canon", bufs=2) as canon_pool,
              tc.tile_pool(name="stage", bufs=2) as stage_pool,
              tc.tile_pool(name="rowmaj", bufs=1) as rowmaj_pool,
              tc.tile_pool(name="gbuf", bufs=2) as gbuf_pool,
              tc.tile_pool(name="idxp", bufs=1) as idx_pool,
              tc.tile_pool(name="wgtp", bufs=2) as wgt_pool,
              tc.tile_pool(name="wexp", bufs=2) as wexp_pool,
              tc.tile_pool(name="t1p", bufs=1) as t1_pool,
              tc.tile_pool(name="vhp", bufs=2) as vh_pool,
              tc.tile_pool(name="btmp", bufs=1) as btmp_pool):

            def merged_copy(out_ap, ps_ap):
                """PSUM->SBUF copy, alternating Scalar / Vector."""
                nonlocal copy_tick
                if copy_tick % 2 == 0:
                    nc.scalar.copy(out=out_ap, in_=ps_ap)
                else:
                    nc.vector.tensor_copy(out_ap, ps_ap)
                copy_tick += 1

            canons = {}

            def emit_load(s):
                canon = canon_pool.tile([128, 2, C, W], dt.bfloat16,
                                        name="canon")
                canons[s] = canon
                # ---- load + cast to bf16 canonical [p=y&127, yt, c, x]
                for yt in range(2):
                    for ch in range(4):
                        stage = stage_pool.tile([128, 6, W], dt.float32,
                                                name="stage")
                        nc.sync.dma_start(
                            out=stage[:, :, :],
                            in_=img_in[s, 6 * ch:6 * (ch + 1),
                                       128 * yt:128 * (yt + 1), :]
                            .rearrange("c p x -> p c x"))
                        nc.scalar.copy(
                            out=canon[:, yt, 6 * ch:6 * (ch + 1), :],
                            in_=stage[:, :, :])

            def emit_slot(s, ty, slot_id, late_hook=None):
                canon = canons[s]
                if ty == "P":
                    p_slot = slot_id
                    vh = vh_pool.tile([128, 2, 2, 256], dt.bfloat16,
                                      name="vh")
                    nc.sync.dma_start(
                        out=vh[:, :, :, :],
                        in_=vh_in[p_slot].rearrange("w p rt f -> p w rt f"))
                    t1 = t1_pool.tile([128, 2, C, 256], dt.bfloat16,
                                      name="t1")
                    # pass 1: T1[c, y] = sum_r I[r, c] V[r, y]
                    for ct in range(2):
                        for q in range(C // 4):
                            ps = psum_pool.tile([128, 1024], dt.float32,
                                                name="ps")
                            for k in range(4):
                                ch = 4 * q + k
                                for rt in range(2):
                                    nc.tensor.matmul(
                                        ps[:, 256 * k:256 * (k + 1)],
                                        canon[:, rt, ch,
                                              128 * ct:128 * (ct + 1)],
                                        vh[:, 0, rt, :],
                                        start=(rt == 0), stop=(rt == 1))
                            merged_copy(
                                t1[:, ct, 4 * q:4 * (q + 1), :],
                                ps[:, :].rearrange("p (c y) -> p c y", c=4))
                    # pass 2: out[y, x] = sum_c T1[c, y] H[c, x]
                    for yt in range(2):
                        for q in range(C // 4):
                            ps2 = psum_pool.tile([128, 1024], dt.float32,
                                                 name="ps")
                            for k in range(4):
                                ch = 4 * q + k
                                for ct in range(2):
                                    nc.tensor.matmul(
                                        ps2[:, 256 * k:256 * (k + 1)],
                                        t1[:, ct, ch,
                                           128 * yt:128 * (yt + 1)],
                                        vh[:, 1, ct, :],
                                        start=(ct == 0), stop=(ct == 1))
                            merged_copy(
                                canon[:, yt, 4 * q:4 * (q + 1), :],
                                ps2[:, :].rearrange("p (c y) -> p c y", c=4))
                    return
                # ---- G slot (rotation gather)
                g_slot = slot_id
                scr = scr_pool.tile([SCRATCH_ELEMS], dt.bfloat16, name="scr")
                rowmaj = rowmaj_pool.tile([128, 2, W, CPAD], dt.bfloat16,
                                          name="rowmaj")
                scr_m = scr[:].rearrange("(b r sc) -> b r sc", b=128, sc=128)
                scr_v = scr_m.rearrange("b (rt p) sc -> p rt b sc", rt=2)
                for rt in range(2):
                    nc.scalar.copy(
                        out=rowmaj[:, rt, :, 0:C],
                        in_=canon[:, rt, :, :].rearrange("p c x -> p x c"))
                    nc.sync.dma_start(
                        out=scr_v[:, rt, :, 0:64],
                        in_=rowmaj[:, rt, :, :]
                        .rearrange("p (b two) c -> p b (two c)", two=2))
                    nc.sync.dma_start(
                        out=scr_v[:, rt, 0:127, 64:128],
                        in_=rowmaj[:, rt, 2:256, :]
                        .rearrange("p (b two) c -> p b (two c)", two=2))
                    nc.sync.dma_start(
                        out=scr_v[:, rt, 127:128, 64:128],
                        in_=rowmaj[:, rt, 0:2, :]
                        .rearrange("p (b two) c -> p b (two c)", two=2))
                idxt = idx_pool.tile([128, NPIX // 16], dt.int16, name="idxt")
                nc.sync.dma_start(out=idxt[:, :], in_=idx_in[g_slot])
                wgtt = wgt_pool.tile([128, 6, NPIX // 128], dt.bfloat16,
                                     name="wgtt")
                nc.sync.dma_start(out=wgtt[:, :, :], in_=wgt_in[g_slot])
                srcv = scr[:]
                srcv.ap = bass_rust.VecI64Pair([(128, 32767), (1, 256)])
                for ck in range(N_CHUNKS):
                    gbuf = gbuf_pool.tile([128, CHUNK_PIX // 128, 256],
                                          dt.bfloat16, name="gbuf")
                    for g in range(CHUNK_PIX // GSUB):
                        i0 = (CHUNK_PIX // 16) * ck + (GSUB // 16) * g
                        nc.gpsimd.dma_gather(
                            gbuf[:, (GSUB // 128) * g:(GSUB // 128) * (g + 1), :],
                            srcv, idxt[:, i0:i0 + GSUB // 16],
                            num_idxs=GSUB, num_idxs_reg=GSUB,
                            elem_size=256, elem_step=128, single_packet=False,
                            queue_num=g % 4)
                    # materialize the 6 per-pixel weight fields expanded
                    # over channels (unit inner stride for the blends)
                    wexp = wexp_pool.tile([128, 6, CHUNK_COLS, 2, C],
                                          dt.bfloat16, name="wexp")
                    wsrc = wgtt[:, :, (CHUNK_PIX // 128) * ck:
                                (CHUNK_PIX // 128) * (ck + 1)] \
                        .rearrange("p t (x yt) -> p t x yt", yt=2).copy()
                    wsrc.ap = bass_rust.VecI64Pair(list(wsrc.ap) + [(0, C)])
                    nc.scalar.copy(out=wexp[:, :, :, :, :], in_=wsrc)
                    acc = btmp_pool.tile([128, CHUNK_COLS, 2, C],
                                         dt.bfloat16, name="acc")
                    tmp = btmp_pool.tile([128, CHUNK_COLS, 2, C],
                                         dt.bfloat16, name="tmp")
                    co = canon[:, :, :,
                               CHUNK_COLS * ck:CHUNK_COLS * (ck + 1)] \
                        .rearrange("p yt c x -> p x yt c")

                    for ti, (r, sl) in enumerate(
                            [(0, 0), (0, 1), (0, 2), (1, 0), (1, 1), (1, 2)]):
                        eview = gbuf[:, :, 128 * r + 32 * sl:
                                     128 * r + 32 * sl + C] \
                            .rearrange("p (x yt) c -> p x yt c", yt=2)
                        if ti == 0:
                            nc.vector.tensor_tensor(
                                acc[:, :, :, :], eview, wexp[:, ti],
                                Alu.mult)
                        else:
                            nc.vector.tensor_tensor(
                                tmp[:, :, :, :], eview, wexp[:, ti],
                                Alu.mult)
                            nc.vector.tensor_tensor(
                                co if ti == 5 else acc[:, :, :, :],
                                acc[:, :, :, :], tmp[:, :, :, :], Alu.add)
                    if late_hook is not None:
                        late_hook(ck)

            def store_step(s, yt, ch, x0, xn):
                stage2 = stage_pool.tile([128, 6, xn], dt.float32,
                                         name="stage2")
                nc.scalar.copy(
                    out=stage2[:, :, :],
                    in_=canons[s][:, yt, 6 * ch:6 * (ch + 1), x0:x0 + xn])
                nc.sync.dma_start(
                    out=out_t[s, 6 * ch:6 * (ch + 1),
                              128 * yt:128 * (yt + 1), x0:x0 + xn]
                    .rearrange("c p x -> p c x"),
                    in_=stage2[:, :, :])

            def emit_store(s, x0=0):
                for yt in range(2):
                    for ch in range(4):
                        store_step(s, yt, ch, x0, W - x0)

            # ---- emission schedule: slot ids in host fill order -----------
            slot_ids = {}
            next_id = {"P": 0, "G": 0}
            for s in range(SAMPLES_PER_CORE):
                for k, ty in enumerate(template[s]):
                    slot_ids[(s, k)] = next_id[ty]
                    next_id[ty] += 1

            # segments end right after each G slot so that interleaving two
            # samples' segments separates their gather groups (the gpsimd
            # engine executes in order; a G slot that depends on its own
            # sample's preceding P slot would otherwise stall the queue).
            def segments(s):
                segs = [[("load", None)]]
                for k, ty in enumerate(template[s]):
                    segs[-1].append((ty, slot_ids[(s, k)]))
                    if ty == "G":
                        segs.append([])
                segs[-1].append(("store", None))
                return [sg for sg in segs if sg]

            # when sample 0's template ends with G, its canon's first
            # x-half is final once that G's chunk 7 blends land; sprinkle
            # the half-0 store into chunks 8-15 so only half-1 remains
            # in the tail.
            split0 = bool(template[0]) and template[0][-1] == "G"
            n_g0 = template[0].count("G")

            def s0_half0_hook(ck):
                if ck < 8 or ck > 15:
                    return
                yt, ch = divmod(ck - 8, 4)
                store_step(0, yt, ch, 0, 128)

            for pair in range(0, SAMPLES_PER_CORE, 2):
                queues = [segments(s) for s in
                          (pair, pair + 1) if s < SAMPLES_PER_CORE]
                g_seen = 0
                si = 0
                while any(queues):
                    qi = si % len(queues)
                    si += 1
                    if not queues[qi]:
                        continue
                    seg = queues[qi].pop(0)
                    s = pair + qi
                    for item, sid in seg:
                        if item == "load":
                            emit_load(s)
                        elif item == "store":
                            emit_store(s, 128 if (s == 0 and split0) else 0)
                        elif item == "G" and s == 0:
                            g_seen += 1
                            hook = s0_half0_hook if (
                                split0 and g_seen == n_g0) else None
                            emit_slot(s, item, sid, late_hook=hook)
                        else:
                            emit_slot(s, item, sid)

    _split_multi_waits(nc)
    nc.compile()
    return nc


def _split_multi_waits(nc):
    """This container's walrus supports one sync wait per instruction; hoist
    extras onto NoOps."""
    import concourse.mybir as mybir
    n = 0
    for fn in nc.m.functions:
        for bb in fn.blocks:
            out = []
            changed = False
            for inst in bb.instructions:
                si = inst.sync_info
                if si is not None and len(si.on_wait) > 1:
                    waits = list(si.on_wait)
                    for wt in waits[:-1]:
                        nop = mybir.InstNoOp(name=f"wsplit-{n}", ins=[], outs=[])
                        n += 1
                        nop.engine = inst.engine
                        nop.sync_info = mybir.SyncInfo(on_update=[], on_wait=[wt])
                        out.append(nop)
                    si.on_wait = [waits[-1]]
                    changed = True
                out.append(inst)
            if changed:
                bb.instructions = out
    return n


# ---------------------------------------------------------------------------
# entry point

def kernel(images, mask_flip, mask_rot90, mask_trans, mask_scale1, mask_rot1,
           mask_scale2, mask_rot2, scale1, angle1, scale2, angle2,
           rot90_k, shift_h, shift_w):
    import ml_dtypes
    from concourse.bass_utils import run_bass_kernel_spmd

    B = images.shape[0]
    imgs = np.ascontiguousarray(images, dtype=np.float32).reshape(B, C, H, W)
    rot90_k = int(rot90_k)
    shift_h = int(shift_h)
    shift_w = int(shift_w)

    all_slots = []
    for i in range(B):
        slots = _sample_slots(
            bool(mask_scale1[i]), float(scale1[i]),
            bool(mask_rot1[i]), float(angle1[i]),
            bool(mask_scale2[i]), np.asarray(scale2[i], dtype=np.float64),
            bool(mask_rot2[i]), float(angle2[i]))
        all_slots.append(slots)

    seqs = [tuple(ty for ty, _ in sl) for sl in all_slots]
    groups = _optimize_groups(seqs)
    template = tuple(
        _min_template([seqs[i] for i in grp])[0] for grp in groups)

    if template not in _GRAPH_CACHE:
        _GRAPH_CACHE[template] = _build_graph(template)
    nc = _GRAPH_CACHE[template]
    NG = sum(t.count("G") for t in template)
    NP_ = sum(t.count("P") for t in template)

    id_idx, id_w = _identity_g()
    id_vh = _identity_p()
    in_maps = []
    placement = np.zeros((N_CORES, SAMPLES_PER_CORE), np.int64)
    for core in range(N_CORES):
        img_c = np.empty((SAMPLES_PER_CORE, C, H, W), np.float32)
        idx_c = np.empty((max(NG, 1), 128, NPIX // 16), np.int16)
        wgt_c = np.empty((max(NG, 1), 128, 6, NPIX // 128), np.float32)
        vh_c = np.empty((max(NP_, 1), 2, 128, 2, 256), np.float32)
        if NG == 0:
            idx_c[0], wgt_c[0] = id_idx, id_w
        if NP_ == 0:
            vh_c[0] = id_vh
        g_slot = 0
        p_slot = 0
        for p in range(SAMPLES_PER_CORE):
            samp = groups[p][core]
            placement[core, p] = samp
            im = imgs[samp]
            # host-applied permutation stages (pure index permutations)
            if mask_flip[samp]:
                im = im[:, :, ::-1]
            if mask_rot90[samp]:
                im = np.rot90(im, k=rot90_k, axes=(-2, -1))
            if mask_trans[samp]:
                im = np.roll(im, (shift_h, shift_w), axis=(-2, -1))
            img_c[p] = im
            slots = list(all_slots[samp])
            for ty in template[p]:
                if slots and slots[0][0] == ty:
                    payload = slots.pop(0)[1]
                else:
                    payload = None
                if ty == "P":
                    vh_c[p_slot] = id_vh if payload is None else _pe_arrays(*payload)
                    p_slot += 1
                else:
                    if payload is None:
                        idx_c[g_slot], wgt_c[g_slot] = id_idx, id_w
                    else:
                        idx_c[g_slot], wgt_c[g_slot] = _gather_arrays(*payload)
                    g_slot += 1
            assert not slots, (samp, template[p], seqs[samp])
        in_maps.append({
            "img": img_c,
            "idx": idx_c,
            "wgt": wgt_c.astype(ml_dtypes.bfloat16),
            "vh": vh_c.astype(ml_dtypes.bfloat16),
        })

    trace = os.environ.get("KTRACE", "0") == "1"
    res = run_bass_kernel_spmd(nc, in_maps, core_ids=list(range(N_CORES)),
                               trace=trace)
    kernel._last_result = res

    out = np.empty((B, C, H, W), np.float32)
    for core in range(N_CORES):
        o = np.asarray(res.results[core]["out"], dtype=np.float32)
        for p in range(SAMPLES_PER_CORE):
            out[placement[core, p]] = o[p]
    return out.reshape(B, 3, 8, H, W)
